# revision 27
# baseline (speedup 1.0000x reference)
"""GumbelSlotSelector Trainium kernel (v3: float32r PE pipeline, 64-partition
matmul outputs).

Math (per row r of B*K rows, D=128, H=64):
  h = relu(x @ W1 + b1);  dlogit = h @ (W2[:,1]-W2[:,0]) + (b2[1]-b2[0])
  decision = 1.0 if dlogit + g1 - g0 > 0 else 0.0,  g_i = -log(-log(clip(u_i)))
  keep_probs = sigmoid(dlogit)
  fixup: rows (of K=64 slots) with no active slot activate their argmax(fix_u) slot.

Sharding: pure data-parallel over batch B=8192 -> 8 cores x 1024 batch rows
(65536 (b,k)-rows of 128 features per core).

All PE work runs in float32r (fp32 streamed at bf16 rate with ~12-bit
mantissa; measured ~1.5e-4 matmul rel err -> ~8 decision flips total).
TRN2 ISA: a K=128 matmul may only write PSUM partitions 0-63, so every
matmul output lives on partitions 0-63.

Per-core dataflow, half-strips of 512 rows (s = strip of 1024, h = half):
  DMA x [128p, 4 x 128d] (f32r) -> 4 PE transposes -> XT psum [128, 512]
  -> DVE copy to SBUF -> mm1 (lhsT=W1 [128,64]) -> ht psum [64, 512]
  -> ACT relu(+b1) -> relu_sb[64*h : 64*h+64, :] packed [128, 512]
  -> one mm2 per strip: the emb window has w2d on rows 0-63 of window col
  2j and rows 64-127 of col 2j+1, extracting both halves' dlogits into
  dl psum partitions 2j, 2j+1 in one 512-col pass.
  dl psum [64, 1024]: partition q = 2j+h, column 512g + n
  <-> original row 32768g + 1024j + 512h + n   (strip s = 32g + j).
  dl columns [0,512) are final after strip 31, so the g=0 final phase
  (gumbel decision + sigmoid + per-group fixup) overlaps strips 32-63.
"""
import sys

sys.path.insert(0, "/opt/trn_rl_repo")
import numpy as np
from contextlib import ExitStack

import concourse.bacc as bacc
import concourse.tile as tile
from concourse import mybir, bass_utils
from concourse.bass_interp import get_hw_module

F32 = mybir.dt.float32
F32R = mybir.dt.float32r
AF = mybir.ActivationFunctionType
ALU = mybir.AluOpType

B, K, D, H = 8192, 64, 128, 64
NCORES = 8
R = (B // NCORES) * K          # 65536 rows per core
SR = 1024                      # strip rows
HS = SR // 2                   # half-strip rows
NSTRIP = R // SR               # 64
CLIP_LO = 1e-10
CLIP_HI = float(np.float32(1.0 - 1e-7))

_CACHE = {}


def _build(repeat=1):
    nc = bacc.Bacc("TRN2", target_bir_lowering=False, debug=False,
                   num_devices=NCORES)
    x_d = nc.dram_tensor("x", [R, D], F32R, kind="ExternalInput")
    gu_d = nc.dram_tensor("gu", [R, 2], F32, kind="ExternalInput")
    fu_d = nc.dram_tensor("fu", [R], F32, kind="ExternalInput")
    w1_d = nc.dram_tensor("w1", [D, H], F32R, kind="ExternalInput")
    emb_d = nc.dram_tensor("emb", [128, 192], F32R, kind="ExternalInput")
    b1_d = nc.dram_tensor("b1c", [128, 1], F32, kind="ExternalInput")
    b2_d = nc.dram_tensor("b2dv", [128, 1], F32, kind="ExternalInput")
    eye_d = nc.dram_tensor("eye", [128, 128], F32R, kind="ExternalInput")
    dec_d = nc.dram_tensor("dec", [R], F32, kind="ExternalOutput")
    keep_d = nc.dram_tensor("keep", [R], F32, kind="ExternalOutput")

    with tile.TileContext(nc) as tc, ExitStack() as ctx:
        cpool = ctx.enter_context(tc.tile_pool(name="const", bufs=1))
        xpool = ctx.enter_context(tc.tile_pool(name="x", bufs=8))
        tpool = ctx.enter_context(tc.tile_pool(name="xt", bufs=6))
        rpool = ctx.enter_context(tc.tile_pool(name="relu", bufs=3))
        fpool = ctx.enter_context(tc.tile_pool(name="fin", bufs=1))
        ps_xt = ctx.enter_context(tc.tile_pool(name="psxt", bufs=4, space="PSUM"))
        ps_ht = ctx.enter_context(tc.tile_pool(name="psht", bufs=2, space="PSUM"))
        ps_dl = ctx.enter_context(tc.tile_pool(name="psdl", bufs=1, space="PSUM"))

        w1_sb = cpool.tile([D, H], F32R)
        nc.sync.dma_start(w1_sb[:], w1_d.ap())
        emb_sb = cpool.tile([128, 192], F32R)
        nc.sync.dma_start(emb_sb[:], emb_d.ap())
        b1_sb = cpool.tile([128, 1], F32)
        nc.sync.dma_start(b1_sb[:], b1_d.ap())
        b2_sb = cpool.tile([128, 1], F32)
        nc.sync.dma_start(b2_sb[:], b2_d.ap())
        eye_sb = cpool.tile([128, 128], F32R)
        nc.sync.dma_start(eye_sb[:], eye_d.ap())

        for _rep in range(repeat):
            dl_ps = ps_dl.tile([64, 1024], F32)

            # --- gumbel preprocessing, independent of the MLP: t1 = g1 - g0
            # in the dl layout (partition q, free 512g + n). 4KB contiguous
            # per (q, g) chunk. Runs during pipeline ramp, on the ACT queue.
            gu_sb = fpool.tile([64, 2048], F32)
            nc.scalar.dma_start(
                gu_sb[:].rearrange("q (g n u) -> q g n u", g=2, u=2),
                gu_d.ap().rearrange("(g q n) u -> q g n u", g=2, q=64),
            )
            fu_sb = fpool.tile([64, 1024], F32)
            nc.scalar.dma_start(
                fu_sb[:].rearrange("q (g n) -> q g n", g=2),
                fu_d.ap().rearrange("(g q n) -> q g n", g=2, q=64))

            gu_v = gu_sb[:].rearrange("q (gn u) -> q gn u", u=2)
            a0 = fpool.tile([64, 1024], F32)
            a1 = fpool.tile([64, 1024], F32)
            nc.vector.tensor_scalar(a0[:], gu_v[:, :, 0], CLIP_LO, CLIP_HI,
                                    op0=ALU.max, op1=ALU.min)
            nc.vector.tensor_scalar(a1[:], gu_v[:, :, 1], CLIP_LO, CLIP_HI,
                                    op0=ALU.max, op1=ALU.min)
            # g_i = -log(-log(u_i)); g0m = log(-log u0) = -g0
            nc.scalar.activation(a0[:], a0[:], AF.Ln)
            nc.scalar.activation(a1[:], a1[:], AF.Ln)
            g0m = fpool.tile([64, 1024], F32)
            g1m = fpool.tile([64, 1024], F32)
            nc.scalar.activation(g0m[:], a0[:], AF.Ln, scale=-1.0)
            nc.scalar.activation(g1m[:], a1[:], AF.Ln, scale=-1.0)
            t1n = fpool.tile([64, 1024], F32)
            nc.vector.tensor_sub(t1n[:], g1m[:], g0m[:])  # g0 - g1

            dec_sb = fpool.tile([64, 1024], F32)
            keep_sb = fpool.tile([64, 1024], F32)

            def final_chunk(g):
                """Decision + keep_probs + fixup for dl columns [512g, 512g+512).
                Groups of 64 original rows are contiguous 64-col blocks."""
                cs = slice(512 * g, 512 * g + 512)
                # decision = (dl + b2d) > (g0 - g1), fused in one pass
                nc.vector.scalar_tensor_tensor(
                    dec_sb[:, cs], dl_ps[:, cs], b2_sb[0:64, 0:1], t1n[:, cs],
                    op0=ALU.add, op1=ALU.is_gt)
                nc.scalar.activation(keep_sb[:, cs], dl_ps[:, cs], AF.Sigmoid,
                                     bias=b2_sb[0:64, 0:1])

                dec_v = dec_sb[:, cs].rearrange("q (b k) -> q b k", k=64)
                fu_v = fu_sb[:, cs].rearrange("q (b k) -> q b k", k=64)
                rs = fpool.tile([64, 8], F32)
                nc.vector.reduce_sum(rs[:], dec_v, axis=mybir.AxisListType.X)
                need = fpool.tile([64, 8], F32)
                nc.vector.tensor_scalar(need[:], rs[:], 0.0, None,
                                        op0=ALU.is_equal)
                fmx = fpool.tile([64, 8], F32)
                nc.vector.reduce_max(fmx[:], fu_v, axis=mybir.AxisListType.X)
                fixm = fpool.tile([64, 512], F32)
                fixm_v = fixm[:].rearrange("q (b k) -> q b k", k=64)
                nc.vector.tensor_tensor(
                    fixm_v[:], fu_v[:],
                    fmx[:].unsqueeze(2).broadcast_to((64, 8, 64)), op=ALU.is_ge)
                nc.vector.tensor_tensor(
                    fixm_v[:], fixm_v[:],
                    need[:].unsqueeze(2).broadcast_to((64, 8, 64)), op=ALU.mult)
                nc.vector.tensor_tensor(dec_sb[:, cs], dec_sb[:, cs], fixm[:],
                                        op=ALU.max)

                nc.sync.dma_start(
                    dec_d.ap().rearrange("(g q n) -> q g n", g=2, q=64)[:, g, :],
                    dec_sb[:, cs])
                nc.sync.dma_start(
                    keep_d.ap().rearrange("(g q n) -> q g n", g=2, q=64)[:, g, :],
                    keep_sb[:, cs])

            for s in range(NSTRIP):
                g, j = s // 32, s % 32
                relu_sb = rpool.tile([128, 512], F32R)
                for h in range(2):
                    x_sb = xpool.tile([128, HS], F32R)
                    r0 = s * SR + h * HS
                    nc.sync.dma_start(
                        x_sb[:].rearrange("p (t d) -> p t d", d=D),
                        x_d.ap()[r0:r0 + HS, :].rearrange(
                            "(t p) d -> p t d", p=128),
                    )
                    xt_ps = ps_xt.tile([128, HS], F32R)
                    for t in range(4):
                        nc.tensor.transpose(
                            xt_ps[:, t * 128:(t + 1) * 128],
                            x_sb[:, t * D:(t + 1) * D],
                            eye_sb[:],
                        )
                    xt_sb = tpool.tile([128, HS], F32R)
                    nc.vector.tensor_copy(xt_sb[:], xt_ps[:])
                    ht_ps = ps_ht.tile([64, HS], F32)
                    nc.tensor.matmul(ht_ps[:], w1_sb[:], xt_sb[:],
                                     start=True, stop=True)
                    nc.scalar.activation(relu_sb[64 * h:64 * h + 64, :],
                                         ht_ps[:], AF.Relu,
                                         bias=b1_sb[64 * h:64 * h + 64, 0:1])

                # mm2: emb window for block j has the w2d column pair at
                # window cols (2j, 2j+1) -> dl partitions 2j, 2j+1
                nc.tensor.matmul(
                    dl_ps[:, 512 * g:512 * g + 512],
                    emb_sb[:, 64 - 2 * j:128 - 2 * j],
                    relu_sb[:],
                    start=(j == 0), stop=(j == 31),
                    skip_group_check=True,
                )
                if s == 33:
                    final_chunk(0)
            final_chunk(1)

    nc.compile()
    nc.m = get_hw_module(nc.m)
    return nc


def _prep_consts(W1, b1, W2, b2):
    w2d = (W2[:, 1] - W2[:, 0]).astype(np.float32)
    b2d = np.float32(b2[1] - b2[0])

    emb = np.zeros((128, 192), np.float32)
    emb[0:64, 64] = w2d
    emb[64:128, 65] = w2d
    b1c = np.concatenate([b1, b1]).astype(np.float32).reshape(128, 1)
    b2dv = np.full((128, 1), b2d, np.float32)
    eye = np.eye(128, dtype=np.float32)
    return emb, b1c, b2dv, eye


def kernel(slots, gumbel_u, fix_u, W1, b1, W2, b2):
    slots = np.ascontiguousarray(slots, np.float32)
    gumbel_u = np.ascontiguousarray(gumbel_u, np.float32)
    fix_u = np.ascontiguousarray(fix_u, np.float32)
    W1 = np.ascontiguousarray(W1, np.float32)
    emb, b1c, b2dv, eye = _prep_consts(W1, np.asarray(b1, np.float32),
                                       np.asarray(W2, np.float32),
                                       np.asarray(b2, np.float32))

    if "nc" not in _CACHE:
        _CACHE["nc"] = _build()
    nc = _CACHE["nc"]

    bpc = B // NCORES
    in_maps = []
    for c in range(NCORES):
        in_maps.append({
            "x": slots[c * bpc:(c + 1) * bpc].reshape(R, D),
            "gu": gumbel_u[c * bpc:(c + 1) * bpc].reshape(R, 2),
            "fu": fix_u[c * bpc:(c + 1) * bpc].reshape(R),
            "w1": W1, "emb": emb, "b1c": b1c, "b2dv": b2dv, "eye": eye,
        })
    res = bass_utils.run_bass_kernel_spmd(
        nc, in_maps, core_ids=list(range(NCORES)))
    _CACHE["last_result"] = res
    _CACHE["in_maps"] = in_maps

    # dec_d flat order is the original row order per core; just reshape.
    dec = np.concatenate(
        [res.results[c]["dec"].reshape(bpc, K) for c in range(NCORES)], axis=0)
    keep = np.concatenate(
        [res.results[c]["keep"].reshape(bpc, K) for c in range(NCORES)], axis=0)
    return dec, keep
